# revision 35
# baseline (speedup 1.0000x reference)
"""Trainium2 Bass kernel for a pre-LN transformer block (B=256, T=200, E=384).

Data-parallel over batch: 8 NeuronCores x 32 batches. Each core runs the full
block (LN1 -> QKV -> causal attention -> proj+residual -> LN2 -> FFN -> residual)
on its batch shard. Matmul operands are bf16 (fp32 PSUM accumulation); softmax,
LayerNorm statistics and the residual stream stay fp32.

Key layout choices:
  - Residual stream token-major: [128 tokens, 384] tiles, 13 tiles per octet
    (8 batches = 1600 tokens), updated in place by both residual adds.
  - LN gains/biases folded into the weight matrices host-side (exact).
  - LN inv-std via DVE bit-trick rsqrt + 2 Newton steps (keeps ScalarE on the
    exp/copy/relu activation table - no LUT reloads).
  - Activations transposed to feature-major via DMA xbar transpose (bf16).
  - Attention: scoresT = K^T-slices @ Q with keys on partitions; odd heads read
    directly from partitions 64-127 via matmul tile_position (no staging).
  - Softmax denominators via column-mask ones matmuls that land broadcast
    across partitions in head-interleaved form; reciprocal_approx_fast on DVE;
    one fused normalize multiply per batch.
  - Causal mask applied as a 0/1 multiply after exp on GpSimd (exp is
    monotone-safe here: |scores| <= ~3).
"""

import numpy as np
import ml_dtypes

B, T, E, F, NH, HS = 256, 200, 384, 1536, 6, 64
NCORES = 8
BPC = B // NCORES          # batches per core = 32
G = 8                      # batches per octet
NOCT = BPC // G            # 4
TOK = G * T                # 1600 tokens per octet
NT = 13                    # token tiles per octet: 12x128 + 1x64
TW = [128] * 12 + [64]     # tile widths
TOKP = NT * 128            # padded token columns in feature-major tiles (1664)
NCH = 4                    # 400-wide column chunks of TOK
CH = TOK // NCH            # 400

_CACHE = {}


def _install_drain_patch():
    """walrus in this container allows only one sem wait on a Drain; split the
    TileContext exit drain into a chain of single-wait drains."""
    import concourse.tile as tile
    import bass_rust
    from concourse.vector_clock import ScopedClock

    if getattr(tile.TileContext, "_drain_patch", False):
        return

    def _patched(self, tick_clock, wait_clock):
        nc = self.nc
        drain_inst = nc.sync.drain()
        wait_clock.add_sem_waits(
            drain_inst.ins, ScopedClock({None: tick_clock.global_clock})
        )
        si = drain_inst.ins.sync_info
        waits = list(si.on_wait) if si is not None else []
        if len(waits) > 1:
            si.on_wait = waits[:1]
            drain_inst.ins.sync_info = si
            for w in waits[1:]:
                d2 = nc.sync.drain()
                d2.ins.sync_info = bass_rust.SyncInfo(on_wait=[w], on_update=[])
        nc.all_engine_barrier()
        assert self.sems is not None
        popped = nc._tile_sem_poison_stack.pop()
        assert popped is self._sem_poison
        nc.clear_and_free_semaphores(list(self.sems.allocated().values()))
        nc.all_engine_barrier()

    tile.TileContext._drain_and_barrier = _patched
    tile.TileContext._drain_patch = True


def _install_wait_split_patch():
    """walrus here supports only one sync-wait per instruction on several
    templates. Split any multi-wait instruction at the BIR-JSON level into a
    chain of single-wait Drain instructions on the same engine, inserted
    immediately before it."""
    import json
    import concourse.bass_utils as bu
    import concourse.bass2jax as b2j

    if getattr(bu, "_wait_split_patch", False):
        return
    orig = bu.compile_bir_kernel

    def patched(bir_json, tmpdir, neff_name="file.neff"):
        d = json.loads(bir_json)
        uid = [0]
        for fn in d.get("functions", []):
            for bb in fn.get("blocks", []):
                new_insts = []
                for ins in bb.get("instructions", []):
                    si = ins.get("sync_info") or {}
                    waits = si.get("on_wait") or []
                    if len(waits) > 1:
                        for w in waits[:-1]:
                            uid[0] += 1
                            new_insts.append({
                                "debug": ins.get("debug", 0),
                                "engine": ins["engine"],
                                "ins": [],
                                "outs": [],
                                "is_reset_sema": False,
                                "name": f"WSPLIT-{uid[0]}",
                                "opcode": "Drain",
                                "sync_info": {"on_update": [],
                                              "on_wait": [w]},
                            })
                        si["on_wait"] = [waits[-1]]
                        ins["sync_info"] = si
                    new_insts.append(ins)
                bb["instructions"] = new_insts
        return orig(json.dumps(d).encode(), tmpdir, neff_name=neff_name)

    bu.compile_bir_kernel = patched
    b2j.compile_bir_kernel = patched
    bu._wait_split_patch = True


RSQRT_MAGIC = 0x5F3759DF


def _build_nc(n_octets=NOCT, loop_reps=None, ablate=(), no_bias=True):
    ablate = set(ablate)
    use_fp8 = "fp8" in ablate  # abandoned: max-err ~2e-2, over the gate
    import concourse.bass as bass
    import concourse.mybir as mybir
    import concourse.tile as tile

    _install_drain_patch()
    f32 = mybir.dt.float32
    i32 = mybir.dt.int32
    bf16 = mybir.dt.bfloat16
    AF = mybir.ActivationFunctionType
    OP = mybir.AluOpType

    nc = bass.Bass("TRN2")

    x_d = nc.dram_tensor("x", [BPC, T, E], f32, kind="ExternalInput")
    wq_d = nc.dram_tensor("wq", [E, E], bf16, kind="ExternalInput")
    wk_d = nc.dram_tensor("wk", [E, E], bf16, kind="ExternalInput")
    wv_d = nc.dram_tensor("wv", [E, E], bf16, kind="ExternalInput")
    wp_d = nc.dram_tensor("wp", [E, E], bf16, kind="ExternalInput")
    ffn_dt = mybir.dt.float8e4 if use_fp8 else bf16
    w1_d = nc.dram_tensor("w1", [E, F], bf16, kind="ExternalInput")
    w2_d = nc.dram_tensor("w2", [F, E], ffn_dt, kind="ExternalInput")
    cq_d = nc.dram_tensor("cq", [E], f32, kind="ExternalInput")
    ck_d = nc.dram_tensor("ck", [E], f32, kind="ExternalInput")
    b1_d = nc.dram_tensor("b1p", [F], f32, kind="ExternalInput")
    bp_d = nc.dram_tensor("bpb", [1, E], bf16, kind="ExternalInput")
    b2_d = nc.dram_tensor("b2b", [1, E], bf16, kind="ExternalInput")
    m0_d = nc.dram_tensor("m0", [128, NH, T], bf16, kind="ExternalInput")
    m1_d = nc.dram_tensor("m1", [72, NH, 72], bf16, kind="ExternalInput")
    or_d = nc.dram_tensor("onr", [1, 128], bf16, kind="ExternalInput")
    y_d = nc.dram_tensor("y", [BPC, T, E], f32, kind="ExternalOutput")

    x_flat = x_d[:].rearrange("b t d -> (b t) d")
    y_flat = y_d[:].rearrange("b t d -> (b t) d")

    from contextlib import ExitStack

    with tile.TileContext(nc) as tc, ExitStack() as es:
        cpool = es.enter_context(tc.tile_pool(name="const", bufs=1))
        spool = es.enter_context(tc.tile_pool(name="work", bufs=1))
        dpool = es.enter_context(tc.tile_pool(name="dbuf", bufs=2))
        hpool = es.enter_context(tc.tile_pool(name="hot", bufs=3))
        mmpool = es.enter_context(tc.tile_pool(name="mm", bufs=4, space="PSUM"))
        papool = es.enter_context(tc.tile_pool(name="pa", bufs=1, space="PSUM"))
        p1pool = es.enter_context(tc.tile_pool(name="ps1", bufs=2, space="PSUM"))

        # ---- constants ----
        wq_s = cpool.tile([128, 3, E], bf16, tag="wq")
        wk_s = cpool.tile([128, 3, E], bf16, tag="wk")
        wv_s = cpool.tile([128, 3, E], bf16, tag="wv")
        wp_s = cpool.tile([128, 3, E], bf16, tag="wp")
        w1_s = cpool.tile([128, 3, F], bf16, tag="w1")
        w2_s = cpool.tile([128, 12, E], ffn_dt, tag="w2")
        for dst, src in ((wq_s, wq_d), (wk_s, wk_d), (wv_s, wv_d), (wp_s, wp_d),
                         (w1_s, w1_d), (w2_s, w2_d)):
            nc.sync.dma_start(dst[:], src[:].rearrange("(ko p) m -> p ko m", p=128))
        cq_s = cpool.tile([128, 3], f32, tag="cq")
        ck_s = cpool.tile([128, 3], f32, tag="ck")
        b1_s = cpool.tile([128, 12], f32, tag="b1")
        nc.sync.dma_start(cq_s[:], cq_d[:].rearrange("(mo p) -> p mo", p=128))
        nc.sync.dma_start(ck_s[:], ck_d[:].rearrange("(mo p) -> p mo", p=128))
        nc.sync.dma_start(b1_s[:], b1_d[:].rearrange("(mo p) -> p mo", p=128))
        bp_s = cpool.tile([1, E], bf16, tag="bp")
        b2_s = cpool.tile([1, E], bf16, tag="b2")
        nc.sync.dma_start(bp_s[:], bp_d[:])
        nc.sync.dma_start(b2_s[:], b2_d[:])
        m0_s = cpool.tile([128, NH, T], bf16, tag="m0")
        m1_s = cpool.tile([72, NH, 72], bf16, tag="m1")
        or_s = cpool.tile([1, 128], bf16, tag="onr")
        nc.sync.dma_start(m0_s[:], m0_d[:])
        nc.sync.dma_start(m1_s[:], m1_d[:])
        nc.sync.dma_start(or_s[:], or_d[:])
        # column-half masks for interleaved softmax denominators
        cmL = cpool.tile([128, 128], bf16, tag="cmL")
        cmR = cpool.tile([128, 128], bf16, tag="cmR")
        nc.vector.memset(cmL[:], 0.0)
        nc.vector.memset(cmL[:, 0:64], 1.0)
        nc.vector.memset(cmR[:], 0.0)
        nc.vector.memset(cmR[:, 64:128], 1.0)

        def layernorm(src_tile, dst_tile):
            """src [128, NT, E] f32 -> dst [128, 3, NT, 128] bf16 normalized,
            k-blocked feature-major-transposable layout (no gain/bias - folded
            into weights). inv-std on DVE (bit-trick rsqrt + 2 Newton steps) -
            keeps ScalarE's LUT on the exp table."""
            stats = spool.tile([128, NT, 6], f32, tag="stats")
            mv = spool.tile([128, NT, 2], f32, tag="mv")
            nc.vector.memset(mv[:], 1.0)
            for i in range(NT):
                w = TW[i]
                nc.vector.bn_stats(stats[:w, i, :], src_tile[:w, i, :])
            for i in range(NT):
                w = TW[i]
                nc.vector.bn_aggr(mv[:w, i, :], stats[:w, i, :])
            t = spool.tile([128, NT], f32, tag="lt")
            y0 = spool.tile([128, NT], f32, tag="ly0")
            p = spool.tile([128, NT], f32, tag="lp")
            r = spool.tile([128, NT], f32, tag="lr")
            av = spool.tile([128, NT], f32, tag="av")
            b0 = spool.tile([128, NT], f32, tag="b0")
            nc.vector.tensor_scalar(t[:], mv[:, :, 1], 1e-5, None, OP.add)
            # seed: y0 = bitcast(MAGIC + ((~i) >> 1)) ~= rsqrt(t)
            nc.vector.tensor_scalar(
                y0[:].bitcast(i32), t[:].bitcast(i32),
                -1, 1, OP.bitwise_xor, OP.arith_shift_right,
            )
            nc.vector.tensor_scalar(
                y0[:].bitcast(i32), y0[:].bitcast(i32),
                RSQRT_MAGIC, None, OP.add,
            )
            for dst in (r, av):  # 2 Newton steps: y <- y*(1.5 - 0.5*t*y^2)
                nc.vector.tensor_tensor(p[:], t[:], y0[:], OP.mult)
                nc.vector.tensor_tensor(p[:], p[:], y0[:], OP.mult)
                nc.vector.tensor_scalar(p[:], p[:], -0.5, 1.5, OP.mult, OP.add)
                nc.vector.tensor_tensor(dst[:], y0[:], p[:], OP.mult)
                y0 = dst
            nc.vector.tensor_tensor(b0[:], mv[:, :, 0], av[:], OP.mult)
            nc.vector.tensor_scalar(b0[:], b0[:], -1.0, None, OP.mult)
            nc.vector.memset(dst_tile[64:128, :, 12, :], 0.0)
            for i in range(NT):
                w = TW[i]
                if i % 2 == 0:
                    nc.vector.tensor_scalar(
                        dst_tile[:w, :, i, :],
                        src_tile[:w, i, :].rearrange("p (k f) -> p k f", k=3),
                        av[:w, i : i + 1], b0[:w, i : i + 1], OP.mult, OP.add,
                    )
                else:
                    nc.scalar.activation(
                        dst_tile[:w, :, i, :],
                        src_tile[:w, i, :].rearrange("p (k f) -> p k f", k=3),
                        AF.Identity,
                        bias=b0[:w, i : i + 1], scale=av[:w, i : i + 1],
                    )

        def transpose_feat(src_tile, dst_tile):
            """src [128, 3, NT, 128] bf16 (token-major, k-blocked) -> dst
            [128, 3, TOKP] bf16 feature-major. One xbar transpose per k block:
            [128, 1664] -> 13 transposed 128-col blocks land as contiguous
            128-token column groups."""
            for k in range(3):
                nc.scalar.dma_start_transpose(
                    dst_tile[:, k, :].rearrange("p (i l) -> p i l", l=128),
                    src_tile[:, k, :, :],
                )

        state = {}

        def front(o):
            """Octet front half: x load, LN1, transpose. Emitted one octet
            ahead (before the previous octet's FFN) so its DVE/DMA work
            overlaps PE-heavy FFN."""
            r0 = o * TOK
            x_oct = dpool.tile([128, NT, E], f32, tag="resid")
            if "load" in ablate:
                nc.vector.memset(x_oct[0:1, 0:1, 0:1], 0.0)
            else:
                nc.sync.dma_start(
                    x_oct[:, 0:12, :],
                    x_flat[r0 : r0 + 1536].rearrange("(g p) d -> p g d", p=128),
                )
                nc.sync.dma_start(x_oct[0:64, 12, :], x_flat[r0 + 1536 : r0 + 1600])
            h_all = spool.tile([128, 3, NT, 128], bf16, tag="h")
            if "ln" in ablate:
                nc.vector.memset(h_all[0:1, 0:1, 0:1], 0.0)
            else:
                layernorm(x_oct, h_all)
            hT = spool.tile([128, 3, TOKP], bf16, tag="hT")
            if "transpose" in ablate:
                nc.vector.memset(hT[0:1, 0:1, 0:1], 0.0)
            else:
                transpose_feat(h_all, hT)
            state[o] = (x_oct, hT)

        def ffn_pieces(o, x_oct, h2T):
            """FFN emission split into pieces that interleave into the next
            octet's attention batches: the FFN matmuls fill PE bubbles while
            the attention dependency chain runs on ACT/DVE. FFN2's residual
            rides the PE (f32r identity matmul) and lands via a ScalarE copy
            to keep DVE free for the attention reciprocals."""
            r0 = o * TOK
            uT = spool.tile([128, 12, TOK],
                            mybir.dt.float8e4 if use_fp8 else bf16, tag="uT")
            pieces = []
            if "ffn1" in ablate:
                nc.vector.memset(uT[0:1, 0:1, 0:1], 0.0)
            else:
                def p_ffn1(c):
                    for m in range(12):
                        pu = mmpool.tile([128, CH], f32, tag="mm")
                        for k in range(3):
                            nc.tensor.matmul(
                                pu[:],
                                w1_s[:, k, 128 * m : 128 * (m + 1)],
                                h2T[:, k, CH * c : CH * (c + 1)],
                                start=(k == 0), stop=(k == 2),
                            )
                        if c < 3:
                            nc.scalar.activation(
                                uT[:, m, CH * c : CH * (c + 1)], pu[:],
                                AF.Relu, bias=b1_s[:, m : m + 1],
                            )
                        else:
                            nc.vector.tensor_scalar(
                                uT[:, m, CH * c : CH * (c + 1)], pu[:],
                                b1_s[:, m : m + 1], 0.0, OP.add, OP.max,
                            )
                for c in range(NCH):
                    pieces.append(lambda c=c: p_ffn1(c))

            def p_ffn2(tiles):
                for i in tiles:
                    w = TW[i]
                    pf = mmpool.tile([128, E], f32, tag="mm")
                    if use_fp8:
                        for k in range(6):
                            nc.tensor.matmul(
                                pf[:w, :],
                                uT[:, 2 * k : 2 * k + 2, 128 * i : 128 * i + w],
                                w2_s[:, 2 * k : 2 * k + 2, :],
                                start=(k == 0), stop=(no_bias and k == 5),
                                perf_mode=mybir.MatmulPerfMode.DoubleRow,
                            )
                    else:
                        for k in range(12):
                            nc.tensor.matmul(
                                pf[:w, :],
                                uT[:, k, 128 * i : 128 * i + w],
                                w2_s[:, k, :],
                                start=(k == 0), stop=(no_bias and k == 11),
                            )
                    if not no_bias:
                        nc.tensor.matmul(
                            pf[:w, :], or_s[0:1, 0:w], b2_s[:],
                            start=False, stop=True,
                        )
                    nc.vector.tensor_tensor(
                        x_oct[:w, i, :], x_oct[:w, i, :], pf[:w, :], OP.add
                    )
            if "ffn2" not in ablate:
                for tiles in (range(0, 4), range(4, 7), range(7, 10),
                              range(10, 13)):
                    pieces.append(lambda t=tiles: p_ffn2(t))

            def p_store():
                if "store" not in ablate:
                    nc.sync.dma_start(
                        y_flat[r0 : r0 + 1536].rearrange(
                            "(g p) d -> p g d", p=128),
                        x_oct[:, 0:12, :],
                    )
                    nc.sync.dma_start(
                        y_flat[r0 + 1536 : r0 + 1600], x_oct[0:64, 12, :])
            pieces.append(p_store)
            return pieces

        pending = []
        loop_cm = None
        if loop_reps is not None:
            loop_cm = tc.For_i(0, loop_reps, 1)
            loop_cm.__enter__()
        def qkv_v(o):
            """QKV + v GEMMs for octet o. Emitted right after the previous
            phase's proj so the PE work overlaps the LN2 DVE chain."""
            x_oct, hT = state.pop(o)
            qT = spool.tile([128, 3, TOK], bf16, tag="qT")
            kT = spool.tile([128, 3, TOK], bf16, tag="kT")
            qk_list = () if "qk" in ablate else ((qT, wq_s, cq_s), (kT, wk_s, ck_s))
            if "qk" in ablate:
                nc.vector.memset(qT[0:1, 0:1, 0:1], 0.0)
                nc.vector.memset(kT[0:1, 0:1, 0:1], 0.0)
            for c in range(NCH):
                for dstT, w_s, c_s in qk_list:
                    for m in range(3):
                        pq = mmpool.tile([128, CH], f32, tag="mm")
                        for k in range(3):
                            nc.tensor.matmul(
                                pq[:],
                                w_s[:, k, 128 * m : 128 * (m + 1)],
                                hT[:, k, CH * c : CH * (c + 1)],
                                start=(k == 0), stop=(k == 2),
                            )
                        nc.scalar.activation(
                            dstT[:, m, CH * c : CH * (c + 1)], pq[:],
                            AF.Identity, bias=c_s[:, m : m + 1],
                        )
            v_all = spool.tile([128, G, 2, E], bf16, tag="v")
            if "v" in ablate:
                nc.vector.memset(v_all[0:1, 0:1, 0:1, 0:1], 0.0)
            for b in (() if "v" in ablate else range(G)):
                for tt in range(2):
                    w = 128 if tt == 0 else 72
                    col = 200 * b + 128 * tt
                    pv = mmpool.tile([128, E], f32, tag="mm")
                    for k in range(3):
                        nc.tensor.matmul(
                            pv[:w, :],
                            hT[:, k, col : col + w],
                            wv_s[:, k, :],
                            start=(k == 0), stop=(k == 2),
                        )
                    nc.vector.tensor_copy(v_all[:w, b, tt, :], pv[:w, :])
            state[o] = (x_oct, qT, kT, v_all)

        for o in range(n_octets):
            r0 = o * TOK
            if o == 0:
                front(0)
                qkv_v(0)
            x_oct, qT, kT, v_all = state.pop(o)

            # ---- attention (next octet's front half interleaved) ----
            attT = spool.tile([128, 3, TOK], bf16, tag="attT")
            if "attn" in ablate:
                nc.vector.memset(attT[0:1, 0:1, 0:1], 0.0)
            if "attn" in ablate and o + 1 < n_octets:
                front(o + 1)
            for b in (() if "attn" in ablate else range(G)):
                if b == 3 and o + 1 < n_octets:
                    front(o + 1)
                c0 = 200 * b
                expT0 = hpool.tile([128, NH, T], bf16, tag="expT0")
                expT1 = hpool.tile([72, NH, 72], bf16, tag="expT1")

                def kslice(j, r, lo, hi):
                    return kT[64 * r : 64 * r + 64, j, c0 + lo : c0 + hi]

                def qslice(j, r, lo, hi):
                    return qT[64 * r : 64 * r + 64, j, c0 + lo : c0 + hi]

                ps_list = []
                for j in range(3):
                    ps = mmpool.tile([128, 2, 256], f32, tag="mm")
                    ps_list.append(ps)
                    if "scores" in ablate:
                        nc.vector.memset(ps[0:1, 0:1, 0:1], 0.0)
                        continue
                    nc.tensor.matmul(
                        ps[:, 0, 0:T],
                        kslice(j, 0, 0, 128),
                        qslice(j, 0, 0, T),
                        start=True, stop=True,
                    )
                    # odd head reads partitions 64-127 via tile_position;
                    # output must stay within a 64-partition half -> split keys
                    for kk in range(2):
                        nc.tensor.matmul(
                            ps[64 * kk : 64 * kk + 64, 1, 0:T],
                            kslice(j, 1, 64 * kk, 64 * kk + 64),
                            qslice(j, 1, 0, T),
                            start=True, stop=True,
                        )
                ps1 = p1pool.tile([72, NH, 72], f32, tag="ps1")
                if "scores" in ablate:
                    nc.vector.memset(ps1[0:1, 0:1, 0:1], 0.0)
                for j in (() if "scores" in ablate else range(3)):
                    h = 2 * j
                    nc.tensor.matmul(
                        ps1[:, h, :],
                        kslice(j, 0, 128, 200),
                        qslice(j, 0, 128, 200),
                        start=True, stop=True,
                    )
                    nc.tensor.matmul(
                        ps1[0:64, h + 1, :],
                        kslice(j, 1, 128, 192),
                        qslice(j, 1, 128, 200),
                        start=True, stop=True,
                    )
                    nc.tensor.matmul(
                        ps1[64:72, h + 1, :],
                        kslice(j, 1, 192, 200),
                        qslice(j, 1, 128, 200),
                        start=True, stop=True,
                    )
                if "exp" in ablate:
                    nc.vector.memset(expT0[0:1, 0:1, 0:1], 0.0)
                    nc.vector.memset(expT1[0:1, 0:1, 0:1], 0.0)
                else:
                    for j in range(3):
                        nc.scalar.activation(
                            expT0[:, 2 * j : 2 * j + 2, :], ps_list[j][:, :, 0:T],
                            AF.Exp,
                        )
                    nc.scalar.activation(expT1[:], ps1[:], AF.Exp)
                # causal mask on DVE: the attention phase is latency-bound
                # and Pool's 2x-slower TT lengthens the per-batch chain
                if "maskpool" in ablate:
                    nc.gpsimd.tensor_tensor(expT0[:], expT0[:], m0_s[:], OP.mult)
                    nc.gpsimd.tensor_tensor(expT1[:], expT1[:], m1_s[:], OP.mult)
                elif "mask" not in ablate:
                    nc.vector.tensor_tensor(expT0[:], expT0[:], m0_s[:], OP.mult)
                    nc.vector.tensor_tensor(expT1[:], expT1[:], m1_s[:], OP.mult)

                # denominators, broadcast across partitions in interleaved
                # head form: rows 0-63 = even head, 64-127 = odd head
                # denominators, broadcast across partitions in interleaved
                # head form: rows 0-63 = even head, 64-127 = odd head. The
                # reciprocal is the expensive step (~7 cyc/elem on DVE), so
                # keep its element count minimal (head pair packed into the
                # partition dim) and emit bf16.
                rsb = spool.tile([128, 3, T], bf16, tag="rsb")
                if "denom" in ablate:
                    nc.vector.memset(rsb[0:1, 0:1, 0:1], 1.0)
                pa = papool.tile([128, 3, 256], f32, tag="pa")
                if "attv" in ablate:
                    nc.vector.memset(pa[0:1, 0:1, 0:1], 0.0)
                for j in range(3):
                    if "denom" not in ablate:
                        dn = mmpool.tile([128, T], f32, tag="mm")
                        nc.tensor.matmul(
                            dn[:], cmL[:], expT0[:, 2 * j, :],
                            start=True, stop=False,
                        )
                        nc.tensor.matmul(
                            dn[:], cmR[:], expT0[:, 2 * j + 1, :],
                            start=False, stop=False,
                        )
                        nc.tensor.matmul(
                            dn[:, 128:200], cmL[0:72, :], expT1[:, 2 * j, :],
                            start=False, stop=False,
                        )
                        nc.tensor.matmul(
                            dn[:, 128:200], cmR[0:72, :], expT1[:, 2 * j + 1, :],
                            start=False, stop=True,
                        )
                        with nc.allow_low_precision(reason="softmax denom bf16"):
                            nc.vector.reciprocal(rsb[:, j, :], dn[:])
                    if "attv" in ablate:
                        continue
                    for r in range(2):
                        h = 2 * j + r
                        nc.tensor.matmul(
                            pa[64 * r : 64 * r + 64, j, 0:T],
                            v_all[0:128, b, 0, 64 * h : 64 * h + 64],
                            expT0[:, h, :],
                            start=True, stop=False,
                        )
                        nc.tensor.matmul(
                            pa[64 * r : 64 * r + 64, j, 128:200],
                            v_all[0:72, b, 1, 64 * h : 64 * h + 64],
                            expT1[:, h, :],
                            start=False, stop=True,
                        )
                # fused normalize: one multiply per batch
                if "norm" not in ablate:
                    nc.vector.tensor_tensor(
                        attT[:, :, c0 : c0 + T], pa[:, :, 0:T], rsb[:], OP.mult
                    )

            # ---- proj + residual (in place into x_oct) ----
            for i in (() if "proj" in ablate else range(NT)):
                w = TW[i]
                pp = mmpool.tile([128, E], f32, tag="mm")
                for k in range(3):
                    nc.tensor.matmul(
                        pp[:w, :],
                        attT[:, k, 128 * i : 128 * i + w],
                        wp_s[:, k, :],
                        start=(k == 0), stop=(no_bias and k == 2),
                    )
                if no_bias:
                    pass
                else:
                    nc.tensor.matmul(
                        pp[:w, :], or_s[0:1, 0:w], bp_s[:],
                        start=False, stop=True,
                    )
                nc.vector.tensor_tensor(
                    x_oct[:w, i, :], x_oct[:w, i, :], pp[:w, :], OP.add
                )

            # next octet's QKV: PE work that overlaps this octet's LN2 DVE chain
            if o + 1 < n_octets:
                qkv_v(o + 1)

            # ---- LN2 + transpose ----
            h2 = spool.tile([128, 3, NT, 128], bf16, tag="h2")
            if "ln" in ablate:
                nc.vector.memset(h2[0:1, 0:1, 0:1], 0.0)
            else:
                layernorm(x_oct, h2)
            h2T = spool.tile([128, 3, TOKP], bf16, tag="hT2")
            if "transpose" in ablate:
                nc.vector.memset(h2T[0:1, 0:1, 0:1], 0.0)
            else:
                transpose_feat(h2, h2T)

            for p in ffn_pieces(o, x_oct, h2T):
                p()

        if loop_cm is not None:
            loop_cm.__exit__(None, None, None)

    return nc


def _prep_inputs(inputs, use_fp8=False):
    """Host-side folding of LN gains/biases into weights. Exact in fp32."""
    bf = ml_dtypes.bfloat16
    f8 = ml_dtypes.float8_e4m3
    x = np.asarray(inputs["x"], np.float32)
    Wq = np.asarray(inputs["Wq"], np.float32)
    Wk = np.asarray(inputs["Wk"], np.float32)
    Wv = np.asarray(inputs["Wv"], np.float32)
    Wp = np.asarray(inputs["Wproj"], np.float32)
    bproj = np.asarray(inputs["bproj"], np.float32)
    W1 = np.asarray(inputs["W1"], np.float32)
    b1 = np.asarray(inputs["b1"], np.float32)
    W2 = np.asarray(inputs["W2"], np.float32)
    b2 = np.asarray(inputs["b2"], np.float32)
    g1 = np.asarray(inputs["g1"], np.float32)
    be1 = np.asarray(inputs["be1"], np.float32)
    g2 = np.asarray(inputs["g2"], np.float32)
    be2 = np.asarray(inputs["be2"], np.float32)

    s = E ** -0.5
    wq_f = (g1[:, None] * Wq) * s
    wk_f = g1[:, None] * Wk
    wv_f = g1[:, None] * Wv
    cq = (be1 @ Wq) * s
    ck = be1 @ Wk
    cv = be1 @ Wv
    bp_f = bproj + cv @ Wp
    w1_f = g2[:, None] * W1
    b1_f = b1 + be2 @ W1

    m0 = np.zeros((128, NH, T), np.float32)
    sidx = np.arange(128)[:, None]
    tidx = np.arange(T)[None, :]
    m0[:, :, :] = (tidx >= sidx)[:, None, :]
    m1 = np.zeros((72, NH, 72), np.float32)
    si = np.arange(72)[:, None]
    ti = np.arange(72)[None, :]
    m1[:, :, :] = (ti >= si)[:, None, :]

    common = {
        "wq": wq_f.astype(bf), "wk": wk_f.astype(bf), "wv": wv_f.astype(bf),
        "wp": Wp.astype(bf),
        "w1": w1_f.astype(bf),
        "w2": W2.astype(f8 if use_fp8 else bf),
        "cq": cq, "ck": ck, "b1p": b1_f,
        "bpb": bp_f.astype(bf).reshape(1, E), "b2b": b2.astype(bf).reshape(1, E),
        "m0": m0.astype(bf), "m1": m1.astype(bf),
        "onr": np.ones((1, 128), bf),
    }
    return x, common


def kernel(**inputs):
    from concourse.bass_utils import run_bass_kernel_spmd

    _install_wait_split_patch()

    x, common = _prep_inputs(inputs)
    no_bias = not (common["bpb"].astype(np.float32).any()
                   or common["b2b"].astype(np.float32).any())
    key = ("nc", no_bias)
    if key not in _CACHE:
        _CACHE[key] = _build_nc(no_bias=no_bias)
    nc = _CACHE[key]
    in_maps = []
    for c in range(NCORES):
        m = dict(common)
        m["x"] = np.ascontiguousarray(x[c * BPC : (c + 1) * BPC])
        in_maps.append(m)
    res = run_bass_kernel_spmd(nc, in_maps, core_ids=list(range(NCORES)))
    out = np.concatenate([res.results[c]["y"] for c in range(NCORES)], axis=0)
    return out.astype(np.float32)


# revision 37
# speedup vs baseline: 1.0145x; 1.0145x over previous
"""Trainium2 Bass kernel for a pre-LN transformer block (B=256, T=200, E=384).

Data-parallel over batch: 8 NeuronCores x 32 batches. Each core runs the full
block (LN1 -> QKV -> causal attention -> proj+residual -> LN2 -> FFN -> residual)
on its batch shard. Matmul operands are bf16 (fp32 PSUM accumulation); softmax,
LayerNorm statistics and the residual stream stay fp32.

Key layout choices:
  - Residual stream token-major: [128 tokens, 384] tiles, 13 tiles per octet
    (8 batches = 1600 tokens), updated in place by both residual adds.
  - LN gains/biases folded into the weight matrices host-side (exact).
  - LN inv-std via DVE bit-trick rsqrt + 2 Newton steps (keeps ScalarE on the
    exp/copy/relu activation table - no LUT reloads).
  - Activations transposed to feature-major via DMA xbar transpose (bf16).
  - Attention: scoresT = K^T-slices @ Q with keys on partitions; odd heads read
    directly from partitions 64-127 via matmul tile_position (no staging).
  - Softmax denominators via column-mask ones matmuls that land broadcast
    across partitions in head-interleaved form; reciprocal_approx_fast on DVE;
    one fused normalize multiply per batch.
  - Causal mask applied as a 0/1 multiply after exp on GpSimd (exp is
    monotone-safe here: |scores| <= ~3).
"""

import numpy as np
import ml_dtypes

B, T, E, F, NH, HS = 256, 200, 384, 1536, 6, 64
NCORES = 8
BPC = B // NCORES          # batches per core = 32
G = 8                      # batches per octet
NOCT = BPC // G            # 4
TOK = G * T                # 1600 tokens per octet
NT = 13                    # token tiles per octet: 12x128 + 1x64
TW = [128] * 12 + [64]     # tile widths
TOKP = NT * 128            # padded token columns in feature-major tiles (1664)
NCH = 4                    # 400-wide column chunks of TOK
CH = TOK // NCH            # 400

_CACHE = {}


def _install_drain_patch():
    """walrus in this container allows only one sem wait on a Drain; split the
    TileContext exit drain into a chain of single-wait drains."""
    import concourse.tile as tile
    import bass_rust
    from concourse.vector_clock import ScopedClock

    if getattr(tile.TileContext, "_drain_patch", False):
        return

    def _patched(self, tick_clock, wait_clock):
        nc = self.nc
        drain_inst = nc.sync.drain()
        wait_clock.add_sem_waits(
            drain_inst.ins, ScopedClock({None: tick_clock.global_clock})
        )
        si = drain_inst.ins.sync_info
        waits = list(si.on_wait) if si is not None else []
        if len(waits) > 1:
            si.on_wait = waits[:1]
            drain_inst.ins.sync_info = si
            for w in waits[1:]:
                d2 = nc.sync.drain()
                d2.ins.sync_info = bass_rust.SyncInfo(on_wait=[w], on_update=[])
        nc.all_engine_barrier()
        assert self.sems is not None
        popped = nc._tile_sem_poison_stack.pop()
        assert popped is self._sem_poison
        nc.clear_and_free_semaphores(list(self.sems.allocated().values()))
        nc.all_engine_barrier()

    tile.TileContext._drain_and_barrier = _patched
    tile.TileContext._drain_patch = True


def _install_wait_split_patch():
    """walrus here supports only one sync-wait per instruction on several
    templates. Split any multi-wait instruction at the BIR-JSON level into a
    chain of single-wait Drain instructions on the same engine, inserted
    immediately before it."""
    import json
    import concourse.bass_utils as bu
    import concourse.bass2jax as b2j

    if getattr(bu, "_wait_split_patch", False):
        return
    orig = bu.compile_bir_kernel

    def patched(bir_json, tmpdir, neff_name="file.neff"):
        d = json.loads(bir_json)
        uid = [0]
        for fn in d.get("functions", []):
            for bb in fn.get("blocks", []):
                new_insts = []
                for ins in bb.get("instructions", []):
                    si = ins.get("sync_info") or {}
                    waits = si.get("on_wait") or []
                    if len(waits) > 1:
                        for w in waits[:-1]:
                            uid[0] += 1
                            new_insts.append({
                                "debug": ins.get("debug", 0),
                                "engine": ins["engine"],
                                "ins": [],
                                "outs": [],
                                "is_reset_sema": False,
                                "name": f"WSPLIT-{uid[0]}",
                                "opcode": "Drain",
                                "sync_info": {"on_update": [],
                                              "on_wait": [w]},
                            })
                        si["on_wait"] = [waits[-1]]
                        ins["sync_info"] = si
                    new_insts.append(ins)
                bb["instructions"] = new_insts
        return orig(json.dumps(d).encode(), tmpdir, neff_name=neff_name)

    bu.compile_bir_kernel = patched
    b2j.compile_bir_kernel = patched
    bu._wait_split_patch = True


RSQRT_MAGIC = 0x5F3759DF


def _build_nc(n_octets=NOCT, loop_reps=None, ablate=(), no_bias=True):
    ablate = set(ablate)
    use_fp8 = "fp8" in ablate  # abandoned: max-err ~2e-2, over the gate
    import concourse.bass as bass
    import concourse.mybir as mybir
    import concourse.tile as tile

    _install_drain_patch()
    f32 = mybir.dt.float32
    i32 = mybir.dt.int32
    bf16 = mybir.dt.bfloat16
    AF = mybir.ActivationFunctionType
    OP = mybir.AluOpType

    nc = bass.Bass("TRN2")

    x_d = nc.dram_tensor("x", [BPC, T, E], f32, kind="ExternalInput")
    wq_d = nc.dram_tensor("wq", [E, E], bf16, kind="ExternalInput")
    wk_d = nc.dram_tensor("wk", [E, E], bf16, kind="ExternalInput")
    wv_d = nc.dram_tensor("wv", [E, E], bf16, kind="ExternalInput")
    wp_d = nc.dram_tensor("wp", [E, E], bf16, kind="ExternalInput")
    ffn_dt = mybir.dt.float8e4 if use_fp8 else bf16
    w1_d = nc.dram_tensor("w1", [E, F], bf16, kind="ExternalInput")
    w2_d = nc.dram_tensor("w2", [F, E], ffn_dt, kind="ExternalInput")
    cq_d = nc.dram_tensor("cq", [E], f32, kind="ExternalInput")
    ck_d = nc.dram_tensor("ck", [E], f32, kind="ExternalInput")
    b1_d = nc.dram_tensor("b1p", [F], f32, kind="ExternalInput")
    bp_d = nc.dram_tensor("bpb", [1, E], bf16, kind="ExternalInput")
    b2_d = nc.dram_tensor("b2b", [1, E], bf16, kind="ExternalInput")
    m0_d = nc.dram_tensor("m0", [128, NH, T], bf16, kind="ExternalInput")
    m1_d = nc.dram_tensor("m1", [72, NH, 72], bf16, kind="ExternalInput")
    or_d = nc.dram_tensor("onr", [1, 128], bf16, kind="ExternalInput")
    y_d = nc.dram_tensor("y", [BPC, T, E], f32, kind="ExternalOutput")

    x_flat = x_d[:].rearrange("b t d -> (b t) d")
    y_flat = y_d[:].rearrange("b t d -> (b t) d")

    from contextlib import ExitStack

    with tile.TileContext(nc) as tc, ExitStack() as es:
        cpool = es.enter_context(tc.tile_pool(name="const", bufs=1))
        spool = es.enter_context(tc.tile_pool(name="work", bufs=1))
        dpool = es.enter_context(tc.tile_pool(name="dbuf", bufs=2))
        hpool = es.enter_context(tc.tile_pool(name="hot", bufs=3))
        mmpool = es.enter_context(tc.tile_pool(name="mm", bufs=3, space="PSUM"))
        papool = es.enter_context(tc.tile_pool(name="pa", bufs=2, space="PSUM"))
        p1pool = es.enter_context(tc.tile_pool(name="ps1", bufs=1, space="PSUM"))

        # ---- constants ----
        wq_s = cpool.tile([128, 3, E], bf16, tag="wq")
        wk_s = cpool.tile([128, 3, E], bf16, tag="wk")
        wv_s = cpool.tile([128, 3, E], bf16, tag="wv")
        wp_s = cpool.tile([128, 3, E], bf16, tag="wp")
        w1_s = cpool.tile([128, 3, F], bf16, tag="w1")
        w2_s = cpool.tile([128, 12, E], ffn_dt, tag="w2")
        for dst, src in ((wq_s, wq_d), (wk_s, wk_d), (wv_s, wv_d), (wp_s, wp_d),
                         (w1_s, w1_d), (w2_s, w2_d)):
            nc.sync.dma_start(dst[:], src[:].rearrange("(ko p) m -> p ko m", p=128))
        cq_s = cpool.tile([128, 3], f32, tag="cq")
        ck_s = cpool.tile([128, 3], f32, tag="ck")
        b1_s = cpool.tile([128, 12], f32, tag="b1")
        nc.sync.dma_start(cq_s[:], cq_d[:].rearrange("(mo p) -> p mo", p=128))
        nc.sync.dma_start(ck_s[:], ck_d[:].rearrange("(mo p) -> p mo", p=128))
        nc.sync.dma_start(b1_s[:], b1_d[:].rearrange("(mo p) -> p mo", p=128))
        bp_s = cpool.tile([1, E], bf16, tag="bp")
        b2_s = cpool.tile([1, E], bf16, tag="b2")
        nc.sync.dma_start(bp_s[:], bp_d[:])
        nc.sync.dma_start(b2_s[:], b2_d[:])
        m0_s = cpool.tile([128, NH, T], bf16, tag="m0")
        m1_s = cpool.tile([72, NH, 72], bf16, tag="m1")
        or_s = cpool.tile([1, 128], bf16, tag="onr")
        nc.sync.dma_start(m0_s[:], m0_d[:])
        nc.sync.dma_start(m1_s[:], m1_d[:])
        nc.sync.dma_start(or_s[:], or_d[:])
        # column-half masks for interleaved softmax denominators
        cmL = cpool.tile([128, 128], bf16, tag="cmL")
        cmR = cpool.tile([128, 128], bf16, tag="cmR")
        nc.vector.memset(cmL[:], 0.0)
        nc.vector.memset(cmL[:, 0:64], 1.0)
        nc.vector.memset(cmR[:], 0.0)
        nc.vector.memset(cmR[:, 64:128], 1.0)

        def layernorm(src_tile, dst_tile):
            """src [128, NT, E] f32 -> dst [128, 3, NT, 128] bf16 normalized,
            k-blocked feature-major-transposable layout (no gain/bias - folded
            into weights). inv-std on DVE (bit-trick rsqrt + 2 Newton steps) -
            keeps ScalarE's LUT on the exp table."""
            stats = spool.tile([128, NT, 6], f32, tag="stats")
            mv = spool.tile([128, NT, 2], f32, tag="mv")
            nc.vector.memset(mv[:], 1.0)
            for i in range(NT):
                w = TW[i]
                nc.vector.bn_stats(stats[:w, i, :], src_tile[:w, i, :])
            for i in range(NT):
                w = TW[i]
                nc.vector.bn_aggr(mv[:w, i, :], stats[:w, i, :])
            t = spool.tile([128, NT], f32, tag="lt")
            y0 = spool.tile([128, NT], f32, tag="ly0")
            p = spool.tile([128, NT], f32, tag="lp")
            r = spool.tile([128, NT], f32, tag="lr")
            av = spool.tile([128, NT], f32, tag="av")
            b0 = spool.tile([128, NT], f32, tag="b0")
            nc.vector.tensor_scalar(t[:], mv[:, :, 1], 1e-5, None, OP.add)
            # seed: y0 = bitcast(MAGIC + ((~i) >> 1)) ~= rsqrt(t)
            nc.vector.tensor_scalar(
                y0[:].bitcast(i32), t[:].bitcast(i32),
                -1, 1, OP.bitwise_xor, OP.arith_shift_right,
            )
            nc.vector.tensor_scalar(
                y0[:].bitcast(i32), y0[:].bitcast(i32),
                RSQRT_MAGIC, None, OP.add,
            )
            for dst in (r, av):  # 2 Newton steps: y <- y*(1.5 - 0.5*t*y^2)
                nc.vector.tensor_tensor(p[:], t[:], y0[:], OP.mult)
                nc.vector.tensor_tensor(p[:], p[:], y0[:], OP.mult)
                nc.vector.tensor_scalar(p[:], p[:], -0.5, 1.5, OP.mult, OP.add)
                nc.vector.tensor_tensor(dst[:], y0[:], p[:], OP.mult)
                y0 = dst
            nc.vector.tensor_tensor(b0[:], mv[:, :, 0], av[:], OP.mult)
            nc.vector.tensor_scalar(b0[:], b0[:], -1.0, None, OP.mult)
            nc.vector.memset(dst_tile[64:128, :, 12, :], 0.0)
            for i in range(NT):
                w = TW[i]
                if i % 4 == 0:
                    nc.vector.tensor_scalar(
                        dst_tile[:w, :, i, :],
                        src_tile[:w, i, :].rearrange("p (k f) -> p k f", k=3),
                        av[:w, i : i + 1], b0[:w, i : i + 1], OP.mult, OP.add,
                    )
                else:
                    nc.scalar.activation(
                        dst_tile[:w, :, i, :],
                        src_tile[:w, i, :].rearrange("p (k f) -> p k f", k=3),
                        AF.Identity,
                        bias=b0[:w, i : i + 1], scale=av[:w, i : i + 1],
                    )

        def transpose_feat(src_tile, dst_tile):
            """src [128, 3, NT, 128] bf16 (token-major, k-blocked) -> dst
            [128, 3, TOKP] bf16 feature-major. One xbar transpose per k block:
            [128, 1664] -> 13 transposed 128-col blocks land as contiguous
            128-token column groups."""
            for k in range(3):
                nc.scalar.dma_start_transpose(
                    dst_tile[:, k, :].rearrange("p (i l) -> p i l", l=128),
                    src_tile[:, k, :, :],
                )

        state = {}

        def front(o):
            """Octet front half: x load, LN1, transpose. Emitted one octet
            ahead (before the previous octet's FFN) so its DVE/DMA work
            overlaps PE-heavy FFN."""
            r0 = o * TOK
            x_oct = dpool.tile([128, NT, E], f32, tag="resid")
            if "load" in ablate:
                nc.vector.memset(x_oct[0:1, 0:1, 0:1], 0.0)
            else:
                nc.sync.dma_start(
                    x_oct[:, 0:12, :],
                    x_flat[r0 : r0 + 1536].rearrange("(g p) d -> p g d", p=128),
                )
                nc.sync.dma_start(x_oct[0:64, 12, :], x_flat[r0 + 1536 : r0 + 1600])
            h_all = spool.tile([128, 3, NT, 128], bf16, tag="h")
            if "ln" in ablate:
                nc.vector.memset(h_all[0:1, 0:1, 0:1], 0.0)
            else:
                layernorm(x_oct, h_all)
            hT = spool.tile([128, 3, TOKP], bf16, tag="hT")
            if "transpose" in ablate:
                nc.vector.memset(hT[0:1, 0:1, 0:1], 0.0)
            else:
                transpose_feat(h_all, hT)
            state[o] = (x_oct, hT)

        def ffn_pieces(o, x_oct, h2T):
            """FFN emission split into pieces that interleave into the next
            octet's attention batches: the FFN matmuls fill PE bubbles while
            the attention dependency chain runs on ACT/DVE. FFN2's residual
            rides the PE (f32r identity matmul) and lands via a ScalarE copy
            to keep DVE free for the attention reciprocals."""
            r0 = o * TOK
            uT = spool.tile([128, 12, TOK],
                            mybir.dt.float8e4 if use_fp8 else bf16, tag="uT")
            pieces = []
            if "ffn1" in ablate:
                nc.vector.memset(uT[0:1, 0:1, 0:1], 0.0)
            else:
                def p_ffn1(c):
                    for m in range(12):
                        pu = mmpool.tile([128, CH], f32, tag="mm")
                        for k in range(3):
                            nc.tensor.matmul(
                                pu[:],
                                w1_s[:, k, 128 * m : 128 * (m + 1)],
                                h2T[:, k, CH * c : CH * (c + 1)],
                                start=(k == 0), stop=(k == 2),
                            )
                        if c < 3:
                            nc.scalar.activation(
                                uT[:, m, CH * c : CH * (c + 1)], pu[:],
                                AF.Relu, bias=b1_s[:, m : m + 1],
                            )
                        else:
                            nc.vector.tensor_scalar(
                                uT[:, m, CH * c : CH * (c + 1)], pu[:],
                                b1_s[:, m : m + 1], 0.0, OP.add, OP.max,
                            )
                for c in range(NCH):
                    pieces.append(lambda c=c: p_ffn1(c))

            def p_ffn2(tiles):
                for i in tiles:
                    w = TW[i]
                    pf = mmpool.tile([128, E], f32, tag="mm")
                    if use_fp8:
                        for k in range(6):
                            nc.tensor.matmul(
                                pf[:w, :],
                                uT[:, 2 * k : 2 * k + 2, 128 * i : 128 * i + w],
                                w2_s[:, 2 * k : 2 * k + 2, :],
                                start=(k == 0), stop=(no_bias and k == 5),
                                perf_mode=mybir.MatmulPerfMode.DoubleRow,
                            )
                    else:
                        for k in range(12):
                            nc.tensor.matmul(
                                pf[:w, :],
                                uT[:, k, 128 * i : 128 * i + w],
                                w2_s[:, k, :],
                                start=(k == 0), stop=(no_bias and k == 11),
                            )
                    if not no_bias:
                        nc.tensor.matmul(
                            pf[:w, :], or_s[0:1, 0:w], b2_s[:],
                            start=False, stop=True,
                        )
                    nc.vector.tensor_tensor(
                        x_oct[:w, i, :], x_oct[:w, i, :], pf[:w, :], OP.add
                    )
            if "ffn2" not in ablate:
                for tiles in (range(0, 4), range(4, 7), range(7, 10),
                              range(10, 13)):
                    pieces.append(lambda t=tiles: p_ffn2(t))

            def p_store():
                if "store" not in ablate:
                    nc.sync.dma_start(
                        y_flat[r0 : r0 + 1536].rearrange(
                            "(g p) d -> p g d", p=128),
                        x_oct[:, 0:12, :],
                    )
                    nc.sync.dma_start(
                        y_flat[r0 + 1536 : r0 + 1600], x_oct[0:64, 12, :])
            pieces.append(p_store)
            return pieces

        pending = []
        loop_cm = None
        if loop_reps is not None:
            loop_cm = tc.For_i(0, loop_reps, 1)
            loop_cm.__enter__()
        def qkv_v(o):
            """QKV + v GEMMs for octet o. Emitted right after the previous
            phase's proj so the PE work overlaps the LN2 DVE chain."""
            x_oct, hT = state.pop(o)
            qT = spool.tile([128, 3, TOK], bf16, tag="qT")
            kT = spool.tile([128, 3, TOK], bf16, tag="kT")
            qk_list = () if "qk" in ablate else ((qT, wq_s, cq_s), (kT, wk_s, ck_s))
            if "qk" in ablate:
                nc.vector.memset(qT[0:1, 0:1, 0:1], 0.0)
                nc.vector.memset(kT[0:1, 0:1, 0:1], 0.0)
            for c in range(NCH):
                for dstT, w_s, c_s in qk_list:
                    for m in range(3):
                        pq = mmpool.tile([128, CH], f32, tag="mm")
                        for k in range(3):
                            nc.tensor.matmul(
                                pq[:],
                                w_s[:, k, 128 * m : 128 * (m + 1)],
                                hT[:, k, CH * c : CH * (c + 1)],
                                start=(k == 0), stop=(k == 2),
                            )
                        nc.scalar.activation(
                            dstT[:, m, CH * c : CH * (c + 1)], pq[:],
                            AF.Identity, bias=c_s[:, m : m + 1],
                        )
            v_all = spool.tile([128, G, 2, E], bf16, tag="v")
            if "v" in ablate:
                nc.vector.memset(v_all[0:1, 0:1, 0:1, 0:1], 0.0)
            for b in (() if "v" in ablate else range(G)):
                for tt in range(2):
                    w = 128 if tt == 0 else 72
                    col = 200 * b + 128 * tt
                    pv = mmpool.tile([128, E], f32, tag="mm")
                    for k in range(3):
                        nc.tensor.matmul(
                            pv[:w, :],
                            hT[:, k, col : col + w],
                            wv_s[:, k, :],
                            start=(k == 0), stop=(k == 2),
                        )
                    nc.vector.tensor_copy(v_all[:w, b, tt, :], pv[:w, :])
            state[o] = (x_oct, qT, kT, v_all)

        for o in range(n_octets):
            r0 = o * TOK
            if o == 0:
                front(0)
                qkv_v(0)
            x_oct, qT, kT, v_all = state.pop(o)

            # ---- attention (next octet's front half interleaved) ----
            attT = spool.tile([128, 3, TOK], bf16, tag="attT")
            if "attn" in ablate:
                nc.vector.memset(attT[0:1, 0:1, 0:1], 0.0)
            if "attn" in ablate and o + 1 < n_octets:
                front(o + 1)
            for b in (() if "attn" in ablate else range(G)):
                if b == 6 and o + 1 < n_octets:
                    front(o + 1)
                c0 = 200 * b
                expT0 = hpool.tile([128, NH, T], bf16, tag="expT0")
                expT1 = hpool.tile([72, NH, 72], bf16, tag="expT1")

                def kslice(j, r, lo, hi):
                    return kT[64 * r : 64 * r + 64, j, c0 + lo : c0 + hi]

                def qslice(j, r, lo, hi):
                    return qT[64 * r : 64 * r + 64, j, c0 + lo : c0 + hi]

                ps_list = []
                for j in range(3):
                    ps = mmpool.tile([128, 2, 256], f32, tag="mm")
                    ps_list.append(ps)
                    if "scores" in ablate:
                        nc.vector.memset(ps[0:1, 0:1, 0:1], 0.0)
                        continue
                    nc.tensor.matmul(
                        ps[:, 0, 0:T],
                        kslice(j, 0, 0, 128),
                        qslice(j, 0, 0, T),
                        start=True, stop=True,
                    )
                    # odd head reads partitions 64-127 via tile_position;
                    # output must stay within a 64-partition half -> split keys
                    for kk in range(2):
                        nc.tensor.matmul(
                            ps[64 * kk : 64 * kk + 64, 1, 0:T],
                            kslice(j, 1, 64 * kk, 64 * kk + 64),
                            qslice(j, 1, 0, T),
                            start=True, stop=True,
                        )
                ps1 = p1pool.tile([72, NH, 72], f32, tag="ps1")
                if "scores" in ablate:
                    nc.vector.memset(ps1[0:1, 0:1, 0:1], 0.0)
                for j in (() if "scores" in ablate else range(3)):
                    h = 2 * j
                    nc.tensor.matmul(
                        ps1[:, h, :],
                        kslice(j, 0, 128, 200),
                        qslice(j, 0, 128, 200),
                        start=True, stop=True,
                    )
                    nc.tensor.matmul(
                        ps1[0:64, h + 1, :],
                        kslice(j, 1, 128, 192),
                        qslice(j, 1, 128, 200),
                        start=True, stop=True,
                    )
                    nc.tensor.matmul(
                        ps1[64:72, h + 1, :],
                        kslice(j, 1, 192, 200),
                        qslice(j, 1, 128, 200),
                        start=True, stop=True,
                    )
                if "exp" in ablate:
                    nc.vector.memset(expT0[0:1, 0:1, 0:1], 0.0)
                    nc.vector.memset(expT1[0:1, 0:1, 0:1], 0.0)
                else:
                    for j in range(3):
                        nc.scalar.activation(
                            expT0[:, 2 * j : 2 * j + 2, :], ps_list[j][:, :, 0:T],
                            AF.Exp,
                        )
                    nc.scalar.activation(expT1[:], ps1[:], AF.Exp)
                # causal mask on DVE: the attention phase is latency-bound
                # and Pool's 2x-slower TT lengthens the per-batch chain
                if "maskpool" in ablate:
                    nc.gpsimd.tensor_tensor(expT0[:], expT0[:], m0_s[:], OP.mult)
                    nc.gpsimd.tensor_tensor(expT1[:], expT1[:], m1_s[:], OP.mult)
                elif "mask" not in ablate:
                    nc.vector.tensor_tensor(expT0[:], expT0[:], m0_s[:], OP.mult)
                    nc.vector.tensor_tensor(expT1[:], expT1[:], m1_s[:], OP.mult)

                # denominators, broadcast across partitions in interleaved
                # head form: rows 0-63 = even head, 64-127 = odd head
                # denominators, broadcast across partitions in interleaved
                # head form: rows 0-63 = even head, 64-127 = odd head. The
                # reciprocal is the expensive step (~7 cyc/elem on DVE), so
                # keep its element count minimal (head pair packed into the
                # partition dim) and emit bf16.
                rsb = spool.tile([128, 3, T], bf16, tag="rsb")
                if "denom" in ablate:
                    nc.vector.memset(rsb[0:1, 0:1, 0:1], 1.0)
                pa = papool.tile([128, 3, 256], f32, tag="pa")
                if "attv" in ablate:
                    nc.vector.memset(pa[0:1, 0:1, 0:1], 0.0)
                for j in range(3):
                    if "denom" not in ablate:
                        dn = mmpool.tile([128, T], f32, tag="mm")
                        nc.tensor.matmul(
                            dn[:], cmL[:], expT0[:, 2 * j, :],
                            start=True, stop=False,
                        )
                        nc.tensor.matmul(
                            dn[:], cmR[:], expT0[:, 2 * j + 1, :],
                            start=False, stop=False,
                        )
                        nc.tensor.matmul(
                            dn[:, 128:200], cmL[0:72, :], expT1[:, 2 * j, :],
                            start=False, stop=False,
                        )
                        nc.tensor.matmul(
                            dn[:, 128:200], cmR[0:72, :], expT1[:, 2 * j + 1, :],
                            start=False, stop=True,
                        )
                        with nc.allow_low_precision(reason="softmax denom bf16"):
                            nc.vector.reciprocal(rsb[:, j, :], dn[:])
                    if "attv" in ablate:
                        continue
                    for r in range(2):
                        h = 2 * j + r
                        nc.tensor.matmul(
                            pa[64 * r : 64 * r + 64, j, 0:T],
                            v_all[0:128, b, 0, 64 * h : 64 * h + 64],
                            expT0[:, h, :],
                            start=True, stop=False,
                        )
                        nc.tensor.matmul(
                            pa[64 * r : 64 * r + 64, j, 128:200],
                            v_all[0:72, b, 1, 64 * h : 64 * h + 64],
                            expT1[:, h, :],
                            start=False, stop=True,
                        )
                # fused normalize: one multiply per batch
                if "norm" not in ablate:
                    nc.vector.tensor_tensor(
                        attT[:, :, c0 : c0 + T], pa[:, :, 0:T], rsb[:], OP.mult
                    )

            # ---- proj + residual (in place into x_oct) ----
            for i in (() if "proj" in ablate else range(NT)):
                w = TW[i]
                pp = mmpool.tile([128, E], f32, tag="mm")
                for k in range(3):
                    nc.tensor.matmul(
                        pp[:w, :],
                        attT[:, k, 128 * i : 128 * i + w],
                        wp_s[:, k, :],
                        start=(k == 0), stop=(no_bias and k == 2),
                    )
                if no_bias:
                    pass
                else:
                    nc.tensor.matmul(
                        pp[:w, :], or_s[0:1, 0:w], bp_s[:],
                        start=False, stop=True,
                    )
                nc.vector.tensor_tensor(
                    x_oct[:w, i, :], x_oct[:w, i, :], pp[:w, :], OP.add
                )

            # next octet's QKV: PE work that overlaps this octet's LN2 DVE chain
            if o + 1 < n_octets:
                qkv_v(o + 1)

            # ---- LN2 + transpose ----
            h2 = spool.tile([128, 3, NT, 128], bf16, tag="h2")
            if "ln" in ablate:
                nc.vector.memset(h2[0:1, 0:1, 0:1], 0.0)
            else:
                layernorm(x_oct, h2)
            h2T = spool.tile([128, 3, TOKP], bf16, tag="hT2")
            if "transpose" in ablate:
                nc.vector.memset(h2T[0:1, 0:1, 0:1], 0.0)
            else:
                transpose_feat(h2, h2T)

            for p in ffn_pieces(o, x_oct, h2T):
                p()

        if loop_cm is not None:
            loop_cm.__exit__(None, None, None)

    return nc


def _prep_inputs(inputs, use_fp8=False):
    """Host-side folding of LN gains/biases into weights. Exact in fp32."""
    bf = ml_dtypes.bfloat16
    f8 = ml_dtypes.float8_e4m3
    x = np.asarray(inputs["x"], np.float32)
    Wq = np.asarray(inputs["Wq"], np.float32)
    Wk = np.asarray(inputs["Wk"], np.float32)
    Wv = np.asarray(inputs["Wv"], np.float32)
    Wp = np.asarray(inputs["Wproj"], np.float32)
    bproj = np.asarray(inputs["bproj"], np.float32)
    W1 = np.asarray(inputs["W1"], np.float32)
    b1 = np.asarray(inputs["b1"], np.float32)
    W2 = np.asarray(inputs["W2"], np.float32)
    b2 = np.asarray(inputs["b2"], np.float32)
    g1 = np.asarray(inputs["g1"], np.float32)
    be1 = np.asarray(inputs["be1"], np.float32)
    g2 = np.asarray(inputs["g2"], np.float32)
    be2 = np.asarray(inputs["be2"], np.float32)

    s = E ** -0.5
    wq_f = (g1[:, None] * Wq) * s
    wk_f = g1[:, None] * Wk
    wv_f = g1[:, None] * Wv
    cq = (be1 @ Wq) * s
    ck = be1 @ Wk
    cv = be1 @ Wv
    bp_f = bproj + cv @ Wp
    w1_f = g2[:, None] * W1
    b1_f = b1 + be2 @ W1

    m0 = np.zeros((128, NH, T), np.float32)
    sidx = np.arange(128)[:, None]
    tidx = np.arange(T)[None, :]
    m0[:, :, :] = (tidx >= sidx)[:, None, :]
    m1 = np.zeros((72, NH, 72), np.float32)
    si = np.arange(72)[:, None]
    ti = np.arange(72)[None, :]
    m1[:, :, :] = (ti >= si)[:, None, :]

    common = {
        "wq": wq_f.astype(bf), "wk": wk_f.astype(bf), "wv": wv_f.astype(bf),
        "wp": Wp.astype(bf),
        "w1": w1_f.astype(bf),
        "w2": W2.astype(f8 if use_fp8 else bf),
        "cq": cq, "ck": ck, "b1p": b1_f,
        "bpb": bp_f.astype(bf).reshape(1, E), "b2b": b2.astype(bf).reshape(1, E),
        "m0": m0.astype(bf), "m1": m1.astype(bf),
        "onr": np.ones((1, 128), bf),
    }
    return x, common


def kernel(**inputs):
    from concourse.bass_utils import run_bass_kernel_spmd

    _install_wait_split_patch()

    x, common = _prep_inputs(inputs)
    no_bias = not (common["bpb"].astype(np.float32).any()
                   or common["b2b"].astype(np.float32).any())
    key = ("nc", no_bias)
    if key not in _CACHE:
        _CACHE[key] = _build_nc(no_bias=no_bias)
    nc = _CACHE[key]
    in_maps = []
    for c in range(NCORES):
        m = dict(common)
        m["x"] = np.ascontiguousarray(x[c * BPC : (c + 1) * BPC])
        in_maps.append(m)
    res = run_bass_kernel_spmd(nc, in_maps, core_ids=list(range(NCORES)))
    out = np.concatenate([res.results[c]["y"] for c in range(NCORES)], axis=0)
    return out.astype(np.float32)


# revision 41
# speedup vs baseline: 1.0391x; 1.0243x over previous
"""Trainium2 Bass kernel for a pre-LN transformer block (B=256, T=200, E=384).

Data-parallel over batch: 8 NeuronCores x 32 batches. Each core runs the full
block (LN1 -> QKV -> causal attention -> proj+residual -> LN2 -> FFN -> residual)
on its batch shard. Matmul operands are bf16 (fp32 PSUM accumulation); softmax,
LayerNorm statistics and the residual stream stay fp32.

Key layout choices:
  - Residual stream token-major: [128 tokens, 384] tiles, 13 tiles per octet
    (8 batches = 1600 tokens), updated in place by both residual adds.
  - LN gains/biases folded into the weight matrices host-side (exact).
  - LN inv-std via DVE bit-trick rsqrt + 2 Newton steps (keeps ScalarE on the
    exp/copy/relu activation table - no LUT reloads).
  - Activations transposed to feature-major via DMA xbar transpose (bf16).
  - Attention: scoresT = K^T-slices @ Q with keys on partitions; odd heads read
    directly from partitions 64-127 via matmul tile_position (no staging).
  - Softmax denominators via column-mask ones matmuls that land broadcast
    across partitions in head-interleaved form; reciprocal_approx_fast on DVE;
    one fused normalize multiply per batch.
  - Causal mask applied as a 0/1 multiply after exp on GpSimd (exp is
    monotone-safe here: |scores| <= ~3).
"""

import numpy as np
import ml_dtypes

B, T, E, F, NH, HS = 256, 200, 384, 1536, 6, 64
NCORES = 8
BPC = B // NCORES          # batches per core = 32
G = 8                      # batches per octet
NOCT = BPC // G            # 4
TOK = G * T                # 1600 tokens per octet
NT = 13                    # token tiles per octet: 12x128 + 1x64
TW = [128] * 12 + [64]     # tile widths
TOKP = NT * 128            # padded token columns in feature-major tiles (1664)
NCH = 4                    # 400-wide column chunks of TOK
CH = TOK // NCH            # 400

_CACHE = {}


def _install_drain_patch():
    """walrus in this container allows only one sem wait on a Drain; split the
    TileContext exit drain into a chain of single-wait drains."""
    import concourse.tile as tile
    import bass_rust
    from concourse.vector_clock import ScopedClock

    if getattr(tile.TileContext, "_drain_patch", False):
        return

    def _patched(self, tick_clock, wait_clock):
        nc = self.nc
        drain_inst = nc.sync.drain()
        wait_clock.add_sem_waits(
            drain_inst.ins, ScopedClock({None: tick_clock.global_clock})
        )
        si = drain_inst.ins.sync_info
        waits = list(si.on_wait) if si is not None else []
        if len(waits) > 1:
            si.on_wait = waits[:1]
            drain_inst.ins.sync_info = si
            for w in waits[1:]:
                d2 = nc.sync.drain()
                d2.ins.sync_info = bass_rust.SyncInfo(on_wait=[w], on_update=[])
        nc.all_engine_barrier()
        assert self.sems is not None
        popped = nc._tile_sem_poison_stack.pop()
        assert popped is self._sem_poison
        nc.clear_and_free_semaphores(list(self.sems.allocated().values()))
        nc.all_engine_barrier()

    tile.TileContext._drain_and_barrier = _patched
    tile.TileContext._drain_patch = True


def _install_wait_split_patch():
    """walrus here supports only one sync-wait per instruction on several
    templates. Split any multi-wait instruction at the BIR-JSON level into a
    chain of single-wait Drain instructions on the same engine, inserted
    immediately before it."""
    import json
    import concourse.bass_utils as bu
    import concourse.bass2jax as b2j

    if getattr(bu, "_wait_split_patch", False):
        return
    orig = bu.compile_bir_kernel

    def patched(bir_json, tmpdir, neff_name="file.neff"):
        d = json.loads(bir_json)
        uid = [0]
        for fn in d.get("functions", []):
            for bb in fn.get("blocks", []):
                new_insts = []
                for ins in bb.get("instructions", []):
                    si = ins.get("sync_info") or {}
                    waits = si.get("on_wait") or []
                    if len(waits) > 1:
                        for w in waits[:-1]:
                            uid[0] += 1
                            new_insts.append({
                                "debug": ins.get("debug", 0),
                                "engine": ins["engine"],
                                "ins": [],
                                "outs": [],
                                "is_reset_sema": False,
                                "name": f"WSPLIT-{uid[0]}",
                                "opcode": "Drain",
                                "sync_info": {"on_update": [],
                                              "on_wait": [w]},
                            })
                        si["on_wait"] = [waits[-1]]
                        ins["sync_info"] = si
                    new_insts.append(ins)
                bb["instructions"] = new_insts
        return orig(json.dumps(d).encode(), tmpdir, neff_name=neff_name)

    bu.compile_bir_kernel = patched
    b2j.compile_bir_kernel = patched
    bu._wait_split_patch = True


RSQRT_MAGIC = 0x5F3759DF


def _build_nc(n_octets=NOCT, loop_reps=None, ablate=(), no_bias=True):
    ablate = set(ablate)
    use_fp8 = "fp8" in ablate  # abandoned: max-err ~2e-2, over the gate
    import concourse.bass as bass
    import concourse.mybir as mybir
    import concourse.tile as tile

    _install_drain_patch()
    f32 = mybir.dt.float32
    i32 = mybir.dt.int32
    bf16 = mybir.dt.bfloat16
    AF = mybir.ActivationFunctionType
    OP = mybir.AluOpType

    nc = bass.Bass("TRN2")

    x_d = nc.dram_tensor("x", [BPC, T, E], f32, kind="ExternalInput")
    wq_d = nc.dram_tensor("wq", [E, E], bf16, kind="ExternalInput")
    wk_d = nc.dram_tensor("wk", [E, E], bf16, kind="ExternalInput")
    wv_d = nc.dram_tensor("wv", [E, E], bf16, kind="ExternalInput")
    wp_d = nc.dram_tensor("wp", [E, E], bf16, kind="ExternalInput")
    ffn_dt = mybir.dt.float8e4 if use_fp8 else bf16
    w1_d = nc.dram_tensor("w1", [E, F], bf16, kind="ExternalInput")
    w2_d = nc.dram_tensor("w2", [F, E], ffn_dt, kind="ExternalInput")
    cq_d = nc.dram_tensor("cq", [E], f32, kind="ExternalInput")
    ck_d = nc.dram_tensor("ck", [E], f32, kind="ExternalInput")
    b1_d = nc.dram_tensor("b1p", [F], f32, kind="ExternalInput")
    bp_d = nc.dram_tensor("bpb", [1, E], bf16, kind="ExternalInput")
    b2_d = nc.dram_tensor("b2b", [1, E], bf16, kind="ExternalInput")
    m0_d = nc.dram_tensor("m0", [128, NH, T], bf16, kind="ExternalInput")
    m1_d = nc.dram_tensor("m1", [72, NH, 72], bf16, kind="ExternalInput")
    or_d = nc.dram_tensor("onr", [1, 128], bf16, kind="ExternalInput")
    y_d = nc.dram_tensor("y", [BPC, T, E], f32, kind="ExternalOutput")

    x_flat = x_d[:].rearrange("b t d -> (b t) d")
    y_flat = y_d[:].rearrange("b t d -> (b t) d")

    from contextlib import ExitStack

    with tile.TileContext(nc) as tc, ExitStack() as es:
        cpool = es.enter_context(tc.tile_pool(name="const", bufs=1))
        spool = es.enter_context(tc.tile_pool(name="work", bufs=1))
        dpool = es.enter_context(tc.tile_pool(name="dbuf", bufs=2))
        hpool = es.enter_context(tc.tile_pool(name="hot", bufs=4))
        mmpool = es.enter_context(tc.tile_pool(name="mm", bufs=3, space="PSUM"))
        papool = es.enter_context(tc.tile_pool(name="pa", bufs=2, space="PSUM"))
        p1pool = es.enter_context(tc.tile_pool(name="ps1", bufs=1, space="PSUM"))

        # ---- constants ----
        wq_s = cpool.tile([128, 3, E], bf16, tag="wq")
        wk_s = cpool.tile([128, 3, E], bf16, tag="wk")
        wv_s = cpool.tile([128, 3, E], bf16, tag="wv")
        wp_s = cpool.tile([128, 3, E], bf16, tag="wp")
        w1_s = cpool.tile([128, 3, F], bf16, tag="w1")
        w2_s = cpool.tile([128, 12, E], ffn_dt, tag="w2")
        for dst, src in ((wq_s, wq_d), (wk_s, wk_d), (wv_s, wv_d), (wp_s, wp_d),
                         (w1_s, w1_d), (w2_s, w2_d)):
            nc.sync.dma_start(dst[:], src[:].rearrange("(ko p) m -> p ko m", p=128))
        cq_s = cpool.tile([128, 3], f32, tag="cq")
        ck_s = cpool.tile([128, 3], f32, tag="ck")
        b1_s = cpool.tile([128, 12], f32, tag="b1")
        nc.sync.dma_start(cq_s[:], cq_d[:].rearrange("(mo p) -> p mo", p=128))
        nc.sync.dma_start(ck_s[:], ck_d[:].rearrange("(mo p) -> p mo", p=128))
        nc.sync.dma_start(b1_s[:], b1_d[:].rearrange("(mo p) -> p mo", p=128))
        bp_s = cpool.tile([1, E], bf16, tag="bp")
        b2_s = cpool.tile([1, E], bf16, tag="b2")
        nc.sync.dma_start(bp_s[:], bp_d[:])
        nc.sync.dma_start(b2_s[:], b2_d[:])
        m0_s = cpool.tile([128, NH, T], bf16, tag="m0")
        m1_s = cpool.tile([72, NH, 72], bf16, tag="m1")
        or_s = cpool.tile([1, 128], bf16, tag="onr")
        nc.sync.dma_start(m0_s[:], m0_d[:])
        nc.sync.dma_start(m1_s[:], m1_d[:])
        nc.sync.dma_start(or_s[:], or_d[:])
        # column-half masks for interleaved softmax denominators
        cmL = cpool.tile([128, 128], bf16, tag="cmL")
        cmR = cpool.tile([128, 128], bf16, tag="cmR")
        nc.vector.memset(cmL[:], 0.0)
        nc.vector.memset(cmL[:, 0:64], 1.0)
        nc.vector.memset(cmR[:], 0.0)
        nc.vector.memset(cmR[:, 64:128], 1.0)

        def layernorm(src_tile, dst_tile):
            """src [128, NT, E] f32 -> dst [128, 3, NT, 128] bf16 normalized,
            k-blocked feature-major-transposable layout (no gain/bias - folded
            into weights). inv-std on DVE (bit-trick rsqrt + 2 Newton steps) -
            keeps ScalarE's LUT on the exp table."""
            stats = spool.tile([128, NT, 6], f32, tag="stats")
            mv = spool.tile([128, NT, 2], f32, tag="mv")
            nc.vector.memset(mv[:], 1.0)
            for i in range(NT):
                w = TW[i]
                nc.vector.bn_stats(stats[:w, i, :], src_tile[:w, i, :])
            for i in range(NT):
                w = TW[i]
                nc.vector.bn_aggr(mv[:w, i, :], stats[:w, i, :])
            t = spool.tile([128, NT], f32, tag="lt")
            y0 = spool.tile([128, NT], f32, tag="ly0")
            p = spool.tile([128, NT], f32, tag="lp")
            r = spool.tile([128, NT], f32, tag="lr")
            av = spool.tile([128, NT], f32, tag="av")
            b0 = spool.tile([128, NT], f32, tag="b0")
            nc.vector.tensor_scalar(t[:], mv[:, :, 1], 1e-5, None, OP.add)
            # seed: y0 = bitcast(MAGIC + ((~i) >> 1)) ~= rsqrt(t)
            nc.vector.tensor_scalar(
                y0[:].bitcast(i32), t[:].bitcast(i32),
                -1, 1, OP.bitwise_xor, OP.arith_shift_right,
            )
            nc.vector.tensor_scalar(
                y0[:].bitcast(i32), y0[:].bitcast(i32),
                RSQRT_MAGIC, None, OP.add,
            )
            for dst in (r, av):  # 2 Newton steps: y <- y*(1.5 - 0.5*t*y^2)
                nc.vector.tensor_tensor(p[:], t[:], y0[:], OP.mult)
                nc.vector.tensor_tensor(p[:], p[:], y0[:], OP.mult)
                nc.vector.tensor_scalar(p[:], p[:], -0.5, 1.5, OP.mult, OP.add)
                nc.vector.tensor_tensor(dst[:], y0[:], p[:], OP.mult)
                y0 = dst
            nc.vector.tensor_tensor(b0[:], mv[:, :, 0], av[:], OP.mult)
            nc.vector.tensor_scalar(b0[:], b0[:], -1.0, None, OP.mult)
            nc.vector.memset(dst_tile[64:128, :, 12, :], 0.0)
            for i in range(NT):
                w = TW[i]
                if i % 2 == 0:
                    nc.vector.tensor_scalar(
                        dst_tile[:w, :, i, :],
                        src_tile[:w, i, :].rearrange("p (k f) -> p k f", k=3),
                        av[:w, i : i + 1], b0[:w, i : i + 1], OP.mult, OP.add,
                    )
                else:
                    nc.scalar.activation(
                        dst_tile[:w, :, i, :],
                        src_tile[:w, i, :].rearrange("p (k f) -> p k f", k=3),
                        AF.Identity,
                        bias=b0[:w, i : i + 1], scale=av[:w, i : i + 1],
                    )

        def transpose_feat(src_tile, dst_tile):
            """src [128, 3, NT, 128] bf16 (token-major, k-blocked) -> dst
            [128, 3, TOKP] bf16 feature-major. One xbar transpose per k block:
            [128, 1664] -> 13 transposed 128-col blocks land as contiguous
            128-token column groups."""
            for k in range(3):
                nc.scalar.dma_start_transpose(
                    dst_tile[:, k, :].rearrange("p (i l) -> p i l", l=128),
                    src_tile[:, k, :, :],
                )

        state = {}

        def front(o):
            """Octet front half: x load, LN1, transpose. Emitted one octet
            ahead (before the previous octet's FFN) so its DVE/DMA work
            overlaps PE-heavy FFN."""
            r0 = o * TOK
            x_oct = dpool.tile([128, NT, E], f32, tag="resid")
            if "load" in ablate:
                nc.vector.memset(x_oct[0:1, 0:1, 0:1], 0.0)
            else:
                nc.sync.dma_start(
                    x_oct[:, 0:12, :],
                    x_flat[r0 : r0 + 1536].rearrange("(g p) d -> p g d", p=128),
                )
                nc.sync.dma_start(x_oct[0:64, 12, :], x_flat[r0 + 1536 : r0 + 1600])
            h_all = spool.tile([128, 3, NT, 128], bf16, tag="h")
            if "ln" in ablate:
                nc.vector.memset(h_all[0:1, 0:1, 0:1], 0.0)
            else:
                layernorm(x_oct, h_all)
            hT = spool.tile([128, 3, TOKP], bf16, tag="hT")
            if "transpose" in ablate:
                nc.vector.memset(hT[0:1, 0:1, 0:1], 0.0)
            else:
                transpose_feat(h_all, hT)
            state[o] = (x_oct, hT)

        def ffn_pieces(o, x_oct, h2T):
            """FFN emission split into pieces that interleave into the next
            octet's attention batches: the FFN matmuls fill PE bubbles while
            the attention dependency chain runs on ACT/DVE. FFN2's residual
            rides the PE (f32r identity matmul) and lands via a ScalarE copy
            to keep DVE free for the attention reciprocals."""
            r0 = o * TOK
            uT = spool.tile([128, 12, TOK],
                            mybir.dt.float8e4 if use_fp8 else bf16, tag="uT")
            pieces = []
            if "ffn1" in ablate:
                nc.vector.memset(uT[0:1, 0:1, 0:1], 0.0)
            else:
                def p_ffn1(c):
                    for m in range(12):
                        pu = mmpool.tile([128, CH], f32, tag="mm")
                        for k in range(3):
                            nc.tensor.matmul(
                                pu[:],
                                w1_s[:, k, 128 * m : 128 * (m + 1)],
                                h2T[:, k, CH * c : CH * (c + 1)],
                                start=(k == 0), stop=(k == 2),
                            )
                        if c < 3:
                            nc.scalar.activation(
                                uT[:, m, CH * c : CH * (c + 1)], pu[:],
                                AF.Relu, bias=b1_s[:, m : m + 1],
                            )
                        else:
                            nc.vector.tensor_scalar(
                                uT[:, m, CH * c : CH * (c + 1)], pu[:],
                                b1_s[:, m : m + 1], 0.0, OP.add, OP.max,
                            )
                for c in range(NCH):
                    pieces.append(lambda c=c: p_ffn1(c))

            def p_ffn2(tiles):
                for i in tiles:
                    w = TW[i]
                    pf = mmpool.tile([128, E], f32, tag="mm")
                    if use_fp8:
                        for k in range(6):
                            nc.tensor.matmul(
                                pf[:w, :],
                                uT[:, 2 * k : 2 * k + 2, 128 * i : 128 * i + w],
                                w2_s[:, 2 * k : 2 * k + 2, :],
                                start=(k == 0), stop=(no_bias and k == 5),
                                perf_mode=mybir.MatmulPerfMode.DoubleRow,
                            )
                    else:
                        for k in range(12):
                            nc.tensor.matmul(
                                pf[:w, :],
                                uT[:, k, 128 * i : 128 * i + w],
                                w2_s[:, k, :],
                                start=(k == 0), stop=(no_bias and k == 11),
                            )
                    if not no_bias:
                        nc.tensor.matmul(
                            pf[:w, :], or_s[0:1, 0:w], b2_s[:],
                            start=False, stop=True,
                        )
                    nc.vector.tensor_tensor(
                        x_oct[:w, i, :], x_oct[:w, i, :], pf[:w, :], OP.add
                    )
            if "ffn2" not in ablate:
                for tiles in (range(0, 4), range(4, 7), range(7, 10),
                              range(10, 13)):
                    pieces.append(lambda t=tiles: p_ffn2(t))

            def p_store():
                if "store" not in ablate:
                    nc.sync.dma_start(
                        y_flat[r0 : r0 + 1536].rearrange(
                            "(g p) d -> p g d", p=128),
                        x_oct[:, 0:12, :],
                    )
                    nc.sync.dma_start(
                        y_flat[r0 + 1536 : r0 + 1600], x_oct[0:64, 12, :])
            pieces.append(p_store)
            return pieces

        pending = []
        loop_cm = None
        if loop_reps is not None:
            loop_cm = tc.For_i(0, loop_reps, 1)
            loop_cm.__enter__()
        def qkv_v(o):
            """QKV + v GEMMs for octet o. Emitted right after the previous
            phase's proj so the PE work overlaps the LN2 DVE chain."""
            x_oct, hT = state.pop(o)
            qT = spool.tile([128, 3, TOK], bf16, tag="qT")
            kT = spool.tile([128, 3, TOK], bf16, tag="kT")
            qk_list = () if "qk" in ablate else ((qT, wq_s, cq_s), (kT, wk_s, ck_s))
            if "qk" in ablate:
                nc.vector.memset(qT[0:1, 0:1, 0:1], 0.0)
                nc.vector.memset(kT[0:1, 0:1, 0:1], 0.0)
            for c in range(NCH):
                for dstT, w_s, c_s in qk_list:
                    for m in range(3):
                        pq = mmpool.tile([128, CH], f32, tag="mm")
                        for k in range(3):
                            nc.tensor.matmul(
                                pq[:],
                                w_s[:, k, 128 * m : 128 * (m + 1)],
                                hT[:, k, CH * c : CH * (c + 1)],
                                start=(k == 0), stop=(k == 2),
                            )
                        nc.scalar.activation(
                            dstT[:, m, CH * c : CH * (c + 1)], pq[:],
                            AF.Identity, bias=c_s[:, m : m + 1],
                        )
            v_all = spool.tile([128, G, 2, E], bf16, tag="v")
            if "v" in ablate:
                nc.vector.memset(v_all[0:1, 0:1, 0:1, 0:1], 0.0)
            for b in (() if "v" in ablate else range(G)):
                for tt in range(2):
                    w = 128 if tt == 0 else 72
                    col = 200 * b + 128 * tt
                    pv = mmpool.tile([128, E], f32, tag="mm")
                    for k in range(3):
                        nc.tensor.matmul(
                            pv[:w, :],
                            hT[:, k, col : col + w],
                            wv_s[:, k, :],
                            start=(k == 0), stop=(k == 2),
                        )
                    nc.vector.tensor_copy(v_all[:w, b, tt, :], pv[:w, :])
            state[o] = (x_oct, qT, kT, v_all)

        for o in range(n_octets):
            r0 = o * TOK
            if o == 0:
                front(0)
                qkv_v(0)
            x_oct, qT, kT, v_all = state.pop(o)

            # ---- attention (next octet's front half interleaved) ----
            attT = spool.tile([128, 3, TOK], bf16, tag="attT")
            if "attn" in ablate:
                nc.vector.memset(attT[0:1, 0:1, 0:1], 0.0)
            if "attn" in ablate and o + 1 < n_octets:
                front(o + 1)
            for b in (() if "attn" in ablate else range(G)):
                if b == 3 and o + 1 < n_octets:
                    front(o + 1)
                c0 = 200 * b
                expT0 = hpool.tile([128, NH, T], bf16, tag="expT0")
                expT1 = hpool.tile([72, NH, 72], bf16, tag="expT1")

                def kslice(j, r, lo, hi):
                    return kT[64 * r : 64 * r + 64, j, c0 + lo : c0 + hi]

                def qslice(j, r, lo, hi):
                    return qT[64 * r : 64 * r + 64, j, c0 + lo : c0 + hi]

                ps_list = []
                for j in range(3):
                    ps = mmpool.tile([128, 2, 256], f32, tag="mm")
                    ps_list.append(ps)
                    if "scores" in ablate:
                        nc.vector.memset(ps[0:1, 0:1, 0:1], 0.0)
                        continue
                    nc.tensor.matmul(
                        ps[:, 0, 0:T],
                        kslice(j, 0, 0, 128),
                        qslice(j, 0, 0, T),
                        start=True, stop=True,
                    )
                    # odd head reads partitions 64-127 via tile_position;
                    # output must stay within a 64-partition half -> split keys
                    for kk in range(2):
                        nc.tensor.matmul(
                            ps[64 * kk : 64 * kk + 64, 1, 0:T],
                            kslice(j, 1, 64 * kk, 64 * kk + 64),
                            qslice(j, 1, 0, T),
                            start=True, stop=True,
                        )
                ps1 = p1pool.tile([72, NH, 72], f32, tag="ps1")
                if "scores" in ablate:
                    nc.vector.memset(ps1[0:1, 0:1, 0:1], 0.0)
                for j in (() if "scores" in ablate else range(3)):
                    h = 2 * j
                    nc.tensor.matmul(
                        ps1[:, h, :],
                        kslice(j, 0, 128, 200),
                        qslice(j, 0, 128, 200),
                        start=True, stop=True,
                    )
                    nc.tensor.matmul(
                        ps1[0:64, h + 1, :],
                        kslice(j, 1, 128, 192),
                        qslice(j, 1, 128, 200),
                        start=True, stop=True,
                    )
                    nc.tensor.matmul(
                        ps1[64:72, h + 1, :],
                        kslice(j, 1, 192, 200),
                        qslice(j, 1, 128, 200),
                        start=True, stop=True,
                    )
                if "exp" in ablate:
                    nc.vector.memset(expT0[0:1, 0:1, 0:1], 0.0)
                    nc.vector.memset(expT1[0:1, 0:1, 0:1], 0.0)
                else:
                    for j in range(3):
                        nc.scalar.activation(
                            expT0[:, 2 * j : 2 * j + 2, :], ps_list[j][:, :, 0:T],
                            AF.Exp,
                        )
                    nc.scalar.activation(expT1[:], ps1[:], AF.Exp)
                # causal mask on DVE: the attention phase is latency-bound
                # and Pool's 2x-slower TT lengthens the per-batch chain
                if "maskpool" in ablate:
                    nc.gpsimd.tensor_tensor(expT0[:], expT0[:], m0_s[:], OP.mult)
                    nc.gpsimd.tensor_tensor(expT1[:], expT1[:], m1_s[:], OP.mult)
                elif "mask" not in ablate:
                    nc.vector.tensor_tensor(expT0[:], expT0[:], m0_s[:], OP.mult)
                    nc.vector.tensor_tensor(expT1[:], expT1[:], m1_s[:], OP.mult)

                # denominators, broadcast across partitions in interleaved
                # head form: rows 0-63 = even head, 64-127 = odd head
                # denominators, broadcast across partitions in interleaved
                # head form: rows 0-63 = even head, 64-127 = odd head. The
                # reciprocal is the expensive step (~7 cyc/elem on DVE), so
                # keep its element count minimal (head pair packed into the
                # partition dim) and emit bf16.
                rsb = spool.tile([128, 3, T], bf16, tag="rsb")
                if "denom" in ablate:
                    nc.vector.memset(rsb[0:1, 0:1, 0:1], 1.0)
                pa = papool.tile([128, 3, 256], f32, tag="pa")
                if "attv" in ablate:
                    nc.vector.memset(pa[0:1, 0:1, 0:1], 0.0)
                for j in range(3):
                    if "denom" not in ablate:
                        dn = mmpool.tile([128, T], f32, tag="mm")
                        nc.tensor.matmul(
                            dn[:], cmL[:], expT0[:, 2 * j, :],
                            start=True, stop=False,
                        )
                        nc.tensor.matmul(
                            dn[:], cmR[:], expT0[:, 2 * j + 1, :],
                            start=False, stop=False,
                        )
                        nc.tensor.matmul(
                            dn[:, 128:200], cmL[0:72, :], expT1[:, 2 * j, :],
                            start=False, stop=False,
                        )
                        nc.tensor.matmul(
                            dn[:, 128:200], cmR[0:72, :], expT1[:, 2 * j + 1, :],
                            start=False, stop=True,
                        )
                        with nc.allow_low_precision(reason="softmax denom bf16"):
                            nc.vector.reciprocal(rsb[:, j, :], dn[:])
                    if "attv" in ablate:
                        continue
                    for r in range(2):
                        h = 2 * j + r
                        nc.tensor.matmul(
                            pa[64 * r : 64 * r + 64, j, 0:T],
                            v_all[0:128, b, 0, 64 * h : 64 * h + 64],
                            expT0[:, h, :],
                            start=True, stop=False,
                        )
                        nc.tensor.matmul(
                            pa[64 * r : 64 * r + 64, j, 128:200],
                            v_all[0:72, b, 1, 64 * h : 64 * h + 64],
                            expT1[:, h, :],
                            start=False, stop=True,
                        )
                # fused normalize: one multiply per batch
                if "norm" not in ablate:
                    nc.vector.tensor_tensor(
                        attT[:, :, c0 : c0 + T], pa[:, :, 0:T], rsb[:], OP.mult
                    )

            # ---- proj + residual (in place into x_oct) ----
            for i in (() if "proj" in ablate else range(NT)):
                w = TW[i]
                pp = mmpool.tile([128, E], f32, tag="mm")
                for k in range(3):
                    nc.tensor.matmul(
                        pp[:w, :],
                        attT[:, k, 128 * i : 128 * i + w],
                        wp_s[:, k, :],
                        start=(k == 0), stop=(no_bias and k == 2),
                    )
                if no_bias:
                    pass
                else:
                    nc.tensor.matmul(
                        pp[:w, :], or_s[0:1, 0:w], bp_s[:],
                        start=False, stop=True,
                    )
                nc.vector.tensor_tensor(
                    x_oct[:w, i, :], x_oct[:w, i, :], pp[:w, :], OP.add
                )

            # next octet's QKV: PE work that overlaps this octet's LN2 DVE chain
            if o + 1 < n_octets:
                qkv_v(o + 1)

            # ---- LN2 + transpose ----
            h2 = spool.tile([128, 3, NT, 128], bf16, tag="h2")
            if "ln" in ablate:
                nc.vector.memset(h2[0:1, 0:1, 0:1], 0.0)
            else:
                layernorm(x_oct, h2)
            h2T = spool.tile([128, 3, TOKP], bf16, tag="hT2")
            if "transpose" in ablate:
                nc.vector.memset(h2T[0:1, 0:1, 0:1], 0.0)
            else:
                transpose_feat(h2, h2T)

            for p in ffn_pieces(o, x_oct, h2T):
                p()

        if loop_cm is not None:
            loop_cm.__exit__(None, None, None)

    return nc


def _prep_inputs(inputs, use_fp8=False):
    """Host-side folding of LN gains/biases into weights. Exact in fp32."""
    bf = ml_dtypes.bfloat16
    f8 = ml_dtypes.float8_e4m3
    x = np.asarray(inputs["x"], np.float32)
    Wq = np.asarray(inputs["Wq"], np.float32)
    Wk = np.asarray(inputs["Wk"], np.float32)
    Wv = np.asarray(inputs["Wv"], np.float32)
    Wp = np.asarray(inputs["Wproj"], np.float32)
    bproj = np.asarray(inputs["bproj"], np.float32)
    W1 = np.asarray(inputs["W1"], np.float32)
    b1 = np.asarray(inputs["b1"], np.float32)
    W2 = np.asarray(inputs["W2"], np.float32)
    b2 = np.asarray(inputs["b2"], np.float32)
    g1 = np.asarray(inputs["g1"], np.float32)
    be1 = np.asarray(inputs["be1"], np.float32)
    g2 = np.asarray(inputs["g2"], np.float32)
    be2 = np.asarray(inputs["be2"], np.float32)

    s = E ** -0.5
    wq_f = (g1[:, None] * Wq) * s
    wk_f = g1[:, None] * Wk
    wv_f = g1[:, None] * Wv
    cq = (be1 @ Wq) * s
    ck = be1 @ Wk
    cv = be1 @ Wv
    bp_f = bproj + cv @ Wp
    w1_f = g2[:, None] * W1
    b1_f = b1 + be2 @ W1

    m0 = np.zeros((128, NH, T), np.float32)
    sidx = np.arange(128)[:, None]
    tidx = np.arange(T)[None, :]
    m0[:, :, :] = (tidx >= sidx)[:, None, :]
    m1 = np.zeros((72, NH, 72), np.float32)
    si = np.arange(72)[:, None]
    ti = np.arange(72)[None, :]
    m1[:, :, :] = (ti >= si)[:, None, :]

    common = {
        "wq": wq_f.astype(bf), "wk": wk_f.astype(bf), "wv": wv_f.astype(bf),
        "wp": Wp.astype(bf),
        "w1": w1_f.astype(bf),
        "w2": W2.astype(f8 if use_fp8 else bf),
        "cq": cq, "ck": ck, "b1p": b1_f,
        "bpb": bp_f.astype(bf).reshape(1, E), "b2b": b2.astype(bf).reshape(1, E),
        "m0": m0.astype(bf), "m1": m1.astype(bf),
        "onr": np.ones((1, 128), bf),
    }
    return x, common


def kernel(**inputs):
    from concourse.bass_utils import run_bass_kernel_spmd

    _install_wait_split_patch()

    x, common = _prep_inputs(inputs)
    no_bias = not (common["bpb"].astype(np.float32).any()
                   or common["b2b"].astype(np.float32).any())
    key = ("nc", no_bias)
    if key not in _CACHE:
        _CACHE[key] = _build_nc(no_bias=no_bias)
    nc = _CACHE[key]
    in_maps = []
    for c in range(NCORES):
        m = dict(common)
        m["x"] = np.ascontiguousarray(x[c * BPC : (c + 1) * BPC])
        in_maps.append(m)
    res = run_bass_kernel_spmd(nc, in_maps, core_ids=list(range(NCORES)))
    out = np.concatenate([res.results[c]["y"] for c in range(NCORES)], axis=0)
    return out.astype(np.float32)
